# revision 13
# baseline (speedup 1.0000x reference)
"""BiGCN (2-layer bidirectional GCN + global add pool) on 8 Trainium2 NeuronCores.

Strategy (hardcoded for the nn_BiGCN_graphcl problem shapes):
  - Nodes are sharded graph-aligned: core c owns graphs [128c, 128c+128) and
    their (contiguous, batch-sorted) node range, padded to a common NPC.
  - Per direction (td / bu), edges are assigned to the core owning their
    target node.  GCNConv is computed as
        out = dinv * (scatter_add(hn[src], dst) + hn) + b,   hn = dinv * (x @ W)
    so no per-edge scaling is needed on device.
  - The hn table ([8*NPC, 128] bf16) is AllGathered between layers; each core
    gathers rows for its edge shard with dma_gather (256B rows), builds a
    staircase one-hot with a DVE is_equal against an iota constant, and
    segment-sums on the TensorEngine into per-window (128-node) PSUM tiles.
  - The SPMD program is identical on all cores: all per-core variation lives
    in uploaded index/data tensors; run lengths are padded to the max across
    cores (pad slots gather row 0 of the block and carry dstloc=-1 so their
    one-hot column is zero).
  - Graph pooling is a second one-hot matmul into a [128 graphs, 128] PSUM
    tile; the host just concatenates the 8 per-core [128, 256] outputs.
"""

import math
import numpy as np
import ml_dtypes

BF16 = ml_dtypes.bfloat16

# ---------------------------------------------------------------- problem cfg
FULL_CFG = dict(
    N=100000, E=1600000, IN_FEATS=256, HIDDEN=128, OUT_FEATS=128,
    NUM_GRAPHS=1024, N_CORES=8, SW=12, NBLK=4,
)


def _round_up(x, m):
    return (x + m - 1) // m * m


# =====================================================================
# Host-side metadata construction
# =====================================================================

def build_partition(batch, cfg):
    """Graph-aligned node partition. Returns dict with per-core node ranges."""
    N, C, G = cfg["N"], cfg["N_CORES"], cfg["NUM_GRAPHS"]
    gpc = G // C  # graphs per core
    # first node of each graph-block boundary
    starts = np.searchsorted(batch, np.arange(0, G + 1, gpc))
    counts = np.diff(starts)
    NPC = max(128, _round_up(int(counts.max()), 128))
    node_core = np.searchsorted(starts[1:], np.arange(N), side="right")
    node_local = np.arange(N) - starts[node_core]
    table_row = node_core * NPC + node_local
    return dict(starts=starts, counts=counts, NPC=NPC, gpc=gpc,
                node_core=node_core.astype(np.int64),
                node_local=node_local.astype(np.int64),
                table_row=table_row.astype(np.int64))


def build_direction_meta(gather_nodes, target_nodes, part, cfg):
    """Build per-core gather index / dstloc arrays and the uniform group
    structure for one edge direction.

    gather_nodes[e]: node whose table row is gathered for edge e.
    target_nodes[e]: node receiving the contribution.
    """
    N, C = cfg["N"], cfg["N_CORES"]
    SW, NBLK = cfg["SW"], cfg["NBLK"]
    NPC = part["NPC"]
    W = NPC // 128
    NS = (W + SW - 1) // SW
    BLK = (C * NPC) // NBLK
    assert BLK <= 32767, f"block size {BLK} exceeds int16 range"

    deg = np.bincount(target_nodes, minlength=N).astype(np.float64) + 1.0

    tr_g = part["table_row"][gather_nodes]
    t_core = part["node_core"][target_nodes]
    t_local = part["node_local"][target_nodes]
    lw = t_local // 128          # window
    dloc = t_local % 128         # position within window
    blk = tr_g // BLK
    idxv = tr_g - blk * BLK
    sup = lw // SW

    # per (core, s, b, w) counts -> uniform G
    keyW = (sup * NBLK + blk) * W + lw  # key within a core
    nkeys = NS * NBLK * W
    counts = np.zeros((C, nkeys), np.int64)
    for c in range(C):
        m = t_core == c
        counts[c] = np.bincount(keyW[m], minlength=nkeys)
    max_counts = counts.max(axis=0).reshape(NS, NBLK, W)

    G = np.ceil(max_counts / 128).astype(np.int64)  # groups per (s,b,w)
    # ensure every window has at least one group (psum must be written)
    for s in range(NS):
        w_lo, w_hi = s * SW, min((s + 1) * SW, W)
        for w in range(w_lo, w_hi):
            if G[s, :, w].sum() == 0:
                G[s, 0, w] = 1
        G[s, :, :w_lo] = 0
        G[s, :, w_hi:] = 0

    # structure: per (s,b): window col bases, totals
    struct = []
    for s in range(NS):
        w_lo, w_hi = s * SW, min((s + 1) * SW, W)
        for b in range(NBLK):
            g_list = G[s, b, w_lo:w_hi]
            base = np.concatenate([[0], np.cumsum(g_list)])
            struct.append(dict(s=s, b=b, w_lo=w_lo, w_hi=w_hi,
                               g_list=g_list, g_base=base,
                               G=int(g_list.sum())))
    # global column offsets
    offG = 0
    off16 = 0
    for sb in struct:
        sb["offG"] = offG
        sb["off16"] = off16
        offG += sb["G"]
        off16 += sb["G"] * 8  # 128 slots / 16
    CG = offG
    Gmax = max((sb["G"] for sb in struct), default=1)

    # per-edge slot assignment (per core)
    idx_all = np.zeros((C, 128, CG * 8), np.int16)
    dloc_all = np.full((C, 128, CG), -1.0, BF16)
    # precompute slot base for each (s,b,w): global slot start
    slot_base = np.zeros((NS, NBLK, W), np.int64)
    for sb in struct:
        s, b = sb["s"], sb["b"]
        for i, w in enumerate(range(sb["w_lo"], sb["w_hi"])):
            slot_base[s, b, w] = (sb["offG"] + sb["g_base"][i]) * 128

    for c in range(C):
        m = t_core == c
        k = keyW[m]
        order = np.argsort(k, kind="stable")
        ks = k[order]
        # rank within each run
        run_start = np.searchsorted(ks, np.arange(nkeys))
        rank = np.arange(len(ks)) - run_start[ks]
        sb_s = ks // (NBLK * W)
        sb_b = (ks // W) % NBLK
        sb_w = ks % W
        slot = slot_base[sb_s, sb_b, sb_w] + rank
        iv = idxv[m][order]
        dv = dloc[m][order]
        # idx wrapped layout: slot j -> (j%16, j//16), replicated x8
        prow = slot % 16
        pcol = slot // 16
        tmp = np.zeros((16, CG * 8), np.int16)
        tmp[prow, pcol] = iv.astype(np.int16)
        idx_all[c] = np.tile(tmp, (8, 1))
        dloc_all[c, slot % 128, slot // 128] = dv.astype(BF16)

    return dict(deg=deg, struct=struct, CG=CG, Gmax=Gmax, NS=NS, W=W,
                BLK=BLK, idx_all=idx_all, dloc_all=dloc_all)


def build_all_inputs(x, edge_index, batch, Ws, bs, cfg):
    """Produce per-core in_maps plus structural metadata."""
    C = cfg["N_CORES"]
    N = cfg["N"]
    part = build_partition(batch, cfg)
    NPC = part["NPC"]
    W = NPC // 128

    src = np.asarray(edge_index[0])
    dst = np.asarray(edge_index[1])
    td = build_direction_meta(src, dst, part, cfg)   # gather src row, scatter to dst
    bu = build_direction_meta(dst, src, part, cfg)   # reversed

    Gmax = max(td["Gmax"], bu["Gmax"])
    iota_rep = np.tile(np.arange(128, dtype=np.float32), Gmax)[None, :].repeat(128, 0).astype(BF16)

    # per-core tensors
    in_maps = []
    xT_full = np.ascontiguousarray(np.asarray(x).T)  # [IN, N]
    batch_np = np.asarray(batch)
    for c in range(C):
        lo, hi = part["starts"][c], part["starts"][c + 1]
        cnt = hi - lo
        xT = np.zeros((cfg["IN_FEATS"], NPC), BF16)
        xT[:, :cnt] = xT_full[:, lo:hi].astype(BF16)
        deg_t = np.ones((128, W), np.float32)
        deg_b = np.ones((128, W), np.float32)
        dt_ = td["deg"][lo:hi].astype(np.float32)
        db_ = bu["deg"][lo:hi].astype(np.float32)
        li = np.arange(cnt)
        deg_t[li % 128, li // 128] = dt_
        deg_b[li % 128, li // 128] = db_
        bl = np.full((128, W), -1.0, BF16)
        bl[li % 128, li // 128] = (batch_np[lo:hi] - c * part["gpc"]).astype(BF16)
        im = dict(
            xT=xT, ident=np.eye(128, dtype=BF16),
            deg_td=deg_t, deg_bu=deg_b, batchloc=bl, iota_rep=iota_rep,
            idx_td=td["idx_all"][c], idx_bu=bu["idx_all"][c],
            dstloc_td=td["dloc_all"][c], dstloc_bu=bu["dloc_all"][c],
            W_td1=Ws[0].astype(BF16), W_bu1=Ws[2].astype(BF16),
            W_td2=Ws[1].astype(BF16), W_bu2=Ws[3].astype(BF16),
            b_td1=np.tile(bs[0][None, :], (128, 1)).astype(np.float32),
            b_td2=np.tile(bs[1][None, :], (128, 1)).astype(np.float32),
            b_bu1=np.tile(bs[2][None, :], (128, 1)).astype(np.float32),
            b_bu2=np.tile(bs[3][None, :], (128, 1)).astype(np.float32),
        )
        in_maps.append(im)
    meta = dict(part=part, td=td, bu=bu, Gmax=Gmax, NPC=NPC, W=W, cfg=cfg)
    return in_maps, meta


# =====================================================================
# Bass program
# =====================================================================

def build_bass(meta):
    import concourse.bacc as bacc
    import concourse.mybir as mybir
    import concourse.tile as tile

    cfg = meta["cfg"]
    C = cfg["N_CORES"]
    NPC, W, Gmax = meta["NPC"], meta["W"], meta["Gmax"]
    IN, HID = cfg["IN_FEATS"], cfg["HIDDEN"]
    NBLK = cfg["NBLK"]
    f32, bf16, i16 = mybir.dt.float32, mybir.dt.bfloat16, mybir.dt.int16

    nc = bacc.Bacc("TRN2", target_bir_lowering=False, debug=False, num_devices=C)

    # ---- I/O ----
    ten = {}
    def inp(name, shape, dt):
        ten[name] = nc.dram_tensor(name, shape, dt, kind="ExternalInput")
        return ten[name]

    inp("xT", [IN, NPC], bf16)
    inp("deg_td", [128, W], f32); inp("deg_bu", [128, W], f32)
    inp("batchloc", [128, W], bf16)
    inp("iota_rep", [128, Gmax * 128], bf16)
    inp("ident", [128, 128], bf16)
    for d in ("td", "bu"):
        m = meta[d]
        inp(f"idx_{d}", [128, m["CG"] * 8], i16)
        inp(f"dstloc_{d}", [128, m["CG"]], bf16)
        inp(f"W_{d}1", [IN, HID], bf16)
        inp(f"W_{d}2", [HID, HID], bf16)
        inp(f"b_{d}1", [128, HID], f32)
        inp(f"b_{d}2", [128, HID], f32)
    out_t = nc.dram_tensor("out", [128, 2 * HID], f32, kind="ExternalOutput")
    dbg = meta.get("dbg")
    if dbg:
        dbg_h1 = {d: nc.dram_tensor(f"dbg_h1_{d}", [NPC, HID], f32, kind="ExternalOutput")
                  for d in ("td", "bu")}
        dbg_m = {d: nc.dram_tensor(f"dbg_m_{d}", [NPC, HID], f32, kind="ExternalOutput")
                 for d in ("td", "bu")}

    # internal DRAM: AG inputs + tables
    ag_in, table = {}, {}
    for d in ("td", "bu"):
        for l in (1, 2):
            ag_in[d, l] = nc.dram_tensor(f"agin_{d}{l}", [NPC, HID], bf16, kind="Internal")
            table[d, l] = nc.dram_tensor(f"table_{d}{l}", [C * NPC, HID], bf16,
                                         kind="Internal", addr_space="Shared")

    rg = [list(range(C))]

    from contextlib import ExitStack
    with tile.TileContext(nc) as tc, ExitStack() as stack:
        def pool(name, bufs, space="SBUF"):
            return stack.enter_context(tc.tile_pool(name=name, bufs=bufs, space=space))

        const = pool("const", 1)
        xt_p = pool("xt", 6)
        hps_p = pool("hps", 2, "PSUM")      # table matmul psum
        hn_p = pool("hn", 4)                 # hn tiles to DRAM
        idx_p = pool("idx", 4)
        dl_p = pool("dl", 4)
        gat_p = pool("gat", 5)               # gathered edge tiles
        oh_p = pool("oh", 3)                 # one-hot tiles
        win_p = pool("win", 4, "PSUM")       # window psum, 4 windows/bank
        epi_p = pool("epi", 6)               # epilogue sbuf tiles
        h1_p = pool("h1", 4)
        t_p = pool("tt", 4)                  # transposes
        pool_ps = pool("plps", 1, "PSUM")    # pooling psum (held whole conv2)
        po_p = pool("po", 4)                 # pool one-hot
        outp = pool("outp", 1)

        # ---- constants in SBUF ----
        iota = const.tile([128, Gmax * 128], bf16, tag="iota")
        nc.sync.dma_start(iota[:], ten["iota_rep"][:])
        Wt = {}
        for d in ("td", "bu"):
            for l, k in ((1, IN), (2, HID)):
                chunks = []
                for kk in range(k // 128):
                    t = const.tile([128, HID], bf16, tag=f"W_{d}{l}_{kk}", name=f"W_{d}{l}_{kk}")
                    nc.sync.dma_start(t[:], ten[f"W_{d}{l}"][kk * 128:(kk + 1) * 128, :])
                    chunks.append(t)
                Wt[d, l] = chunks
        bt = {}
        for d in ("td", "bu"):
            for l in (1, 2):
                t = const.tile([128, HID], f32, tag=f"b_{d}{l}", name=f"bt_{d}{l}")
                nc.sync.dma_start(t[:], ten[f"b_{d}{l}"][:])
                bt[d, l] = t
        zrow = const.tile([1, 512], bf16, tag="zrow")
        nc.gpsimd.memset(zrow[:], 0.0)
        ident = const.tile([128, 128], bf16, tag="ident")
        nc.sync.dma_start(ident[:], ten["ident"][:])
        batchloc = const.tile([128, W], bf16, tag="batchloc")
        nc.sync.dma_start(batchloc[:], ten["batchloc"][:])

        dinv = {}
        for d in ("td", "bu"):
            degt = const.tile([128, W], f32, tag=f"deg_{d}", name=f"degt_{d}")
            nc.sync.dma_start(degt[:], ten[f"deg_{d}"][:])
            rec = const.tile([128, W], f32, tag=f"rec_{d}", name=f"rec_{d}")
            nc.vector.reciprocal(rec[:], degt[:])
            dv = const.tile([128, W], f32, tag=f"dinv_{d}", name=f"dinv_{d}")
            nc.scalar.activation(dv[:], rec[:], mybir.ActivationFunctionType.Sqrt)
            dinv[d] = dv

        # ---- phase A1: conv1 tables (both directions share xT loads) ----
        nK = IN // 128
        for w in range(W):
            xts = []
            for kk in range(nK):
                t = xt_p.tile([128, 128], bf16, tag="xt", name=f"xt_{w}_{kk}")
                nc.sync.dma_start(t[:], ten["xT"][kk * 128:(kk + 1) * 128,
                                                 w * 128:(w + 1) * 128])
                xts.append(t)
            for d in ("td", "bu"):
                hps = hps_p.tile([128, HID], f32, tag="hps")
                for kk in range(nK):
                    nc.tensor.matmul(hps[:], xts[kk][:], Wt[d, 1][kk][:],
                                     start=(kk == 0), stop=(kk == nK - 1))
                hn = hn_p.tile([128, HID], bf16, tag="hn")
                nc.vector.tensor_scalar_mul(hn[:], hps[:], dinv[d][:, w:w + 1])
                nc.sync.dma_start(ag_in[d, 1][w * 128:(w + 1) * 128, :], hn[:])

        for d in ("td", "bu"):
            nc.gpsimd.collective_compute(
                "AllGather", mybir.AluOpType.bypass, replica_groups=rg,
                ins=[ag_in[d, 1].ap()], outs=[table[d, 1].ap()])

        # ---- edge phase for one conv ----
        def edge_phase(d, l):
            m = meta[d]
            first_mm = {}
            last_mm = {}
            # find last (sb_idx, group) per window for stop flags
            for sbi, sb in enumerate(m["struct"]):
                for i, w in enumerate(range(sb["w_lo"], sb["w_hi"])):
                    if sb["g_list"][i] > 0:
                        last_mm[w] = (sbi, int(sb["g_base"][i]) + int(sb["g_list"][i]) - 1)
            quad_tiles = {}
            def win_ap(w):
                q = w // 4
                if q not in quad_tiles:
                    qt = win_p.tile([128, 512], f32, tag="win",
                                    name=f"win_{d}{l}_{q}")
                    nc.tensor.matmul(qt[:], zrow[0:1, 0:128], zrow[0:1, 0:512],
                                     start=True, stop=False, skip_group_check=True)
                    quad_tiles[q] = qt
                return quad_tiles[q][:, (w % 4) * 128:(w % 4 + 1) * 128]
            for sbi, sb in enumerate(m["struct"]):
                G = sb["G"]
                if G == 0:
                    continue
                it = idx_p.tile([128, G * 8], i16, tag="idx")
                nc.sync.dma_start(it[:], ten[f"idx_{d}"][:, sb["off16"]:sb["off16"] + G * 8])
                dlt = dl_p.tile([128, G], bf16, tag="dl")
                nc.sync.dma_start(dlt[:], ten[f"dstloc_{d}"][:, sb["offG"]:sb["offG"] + G])
                gt = gat_p.tile([128, G, 128], bf16, tag="gat")
                blk = table[d, l][sb["b"] * m["BLK"]:(sb["b"] + 1) * m["BLK"], :]
                nc.gpsimd.dma_gather(gt[:], blk, it[:], num_idxs=G * 128,
                                     num_idxs_reg=G * 128, elem_size=HID,
                                     single_packet=False)
                oh = oh_p.tile([128, G * 128], bf16, tag="oh")
                nc.vector.tensor_tensor(
                    out=oh[:],
                    in0=dlt[:].rearrange("p (g o) -> p g o", o=1).to_broadcast([128, G, 128]),
                    in1=iota[:, :G * 128].rearrange("p (g f) -> p g f", f=128),
                    op=mybir.AluOpType.is_equal)
                for i, w in enumerate(range(sb["w_lo"], sb["w_hi"])):
                    gl = int(sb["g_list"][i])
                    if gl == 0:
                        continue
                    pt = win_ap(w)
                    gb = int(sb["g_base"][i])
                    for g in range(gb, gb + gl):
                        nc.tensor.matmul(
                            pt[:], oh[:, g * 128:(g + 1) * 128], gt[:, g, :],
                            start=False, stop=(last_mm[w] == (sbi, g)),
                            skip_group_check=True)
                # epilogues for completed supers: after last block of super
                if sb["b"] == NBLK - 1:
                    for w in range(sb["w_lo"], sb["w_hi"]):
                        epilogue(d, l, w, win_ap(w))
                    quad_tiles.clear()

        def epilogue(d, l, w, pt):
            hn = hn_p.tile([128, HID], bf16, tag="hn_ep")
            nc.sync.dma_start(hn[:], ag_in[d, l][w * 128:(w + 1) * 128, :])
            o1 = epi_p.tile([128, HID], f32, tag="o1")
            nc.vector.scalar_tensor_tensor(
                out=o1[:], in0=pt[:], scalar=dinv[d][:, w:w + 1], in1=bt[d, l][:],
                op0=mybir.AluOpType.mult, op1=mybir.AluOpType.add)
            o2 = epi_p.tile([128, HID], bf16, tag="o2")
            nc.vector.scalar_tensor_tensor(
                out=o2[:], in0=hn[:], scalar=dinv[d][:, w:w + 1], in1=o1[:],
                op0=mybir.AluOpType.mult, op1=mybir.AluOpType.add)
            if dbg and l == 1:
                mf = epi_p.tile([128, HID], f32, tag="mf")
                nc.vector.tensor_copy(mf[:], pt[:])
                nc.sync.dma_start(dbg_m[d][w * 128:(w + 1) * 128, :], mf[:])
            if l == 1:
                h1 = h1_p.tile([128, HID], bf16, tag="h1")
                nc.scalar.activation(h1[:], o2[:], mybir.ActivationFunctionType.Relu)
                if dbg:
                    h1f = epi_p.tile([128, HID], f32, tag="h1f")
                    nc.vector.tensor_copy(h1f[:], h1[:])
                    nc.sync.dma_start(dbg_h1[d][w * 128:(w + 1) * 128, :], h1f[:])
                tps = hps_p.tile([128, HID], bf16, tag="hps", name=f"tps_{d}_{w}")
                nc.tensor.transpose(tps[:], h1[:], ident[:])
                h1T = t_p.tile([128, HID], bf16, tag="h1T")
                nc.vector.tensor_copy(h1T[:], tps[:])
                h2 = hps_p.tile([128, HID], f32, tag="hps")
                nc.tensor.matmul(h2[:], h1T[:], Wt[d, 2][0][:], start=True, stop=True)
                hn2 = hn_p.tile([128, HID], bf16, tag="hn2")
                nc.vector.tensor_scalar_mul(hn2[:], h2[:], dinv[d][:, w:w + 1])
                nc.sync.dma_start(ag_in[d, 2][w * 128:(w + 1) * 128, :], hn2[:])
            else:
                po = po_p.tile([128, 128], bf16, tag="po")
                nc.vector.tensor_tensor(
                    out=po[:],
                    in0=batchloc[:, w:w + 1].to_broadcast([128, 128]),
                    in1=iota[:, :128],
                    op=mybir.AluOpType.is_equal)
                off = 0 if d == "td" else HID
                nc.tensor.matmul(pool_psum_t[:, off:off + HID], po[:], o2[:],
                                 start=False, stop=(w == W - 1),
                                 skip_group_check=True)

        # conv1 td -> AG2 td; conv1 bu -> AG2 bu; conv2 td; conv2 bu
        edge_phase("td", 1)
        nc.gpsimd.collective_compute(
            "AllGather", mybir.AluOpType.bypass, replica_groups=rg,
            ins=[ag_in["td", 2].ap()], outs=[table["td", 2].ap()])
        edge_phase("bu", 1)
        nc.gpsimd.collective_compute(
            "AllGather", mybir.AluOpType.bypass, replica_groups=rg,
            ins=[ag_in["bu", 2].ap()], outs=[table["bu", 2].ap()])
        pool_psum_t = pool_ps.tile([128, 2 * HID], f32, tag="pool", name="pool_psum_t")
        nc.tensor.matmul(pool_psum_t[:], zrow[0:1, 0:128], zrow[0:1, 0:2 * HID],
                         start=True, stop=False, skip_group_check=True)
        edge_phase("td", 2)
        edge_phase("bu", 2)

        outsb = outp.tile([128, 2 * HID], f32, tag="out")
        nc.vector.tensor_copy(outsb[:], pool_psum_t[:])
        nc.sync.dma_start(out_t[:], outsb[:])

    nc.compile()
    return nc


# =====================================================================
# Entry point
# =====================================================================

def _run(inputs, cfg, trace=False):
    from concourse import bass_utils
    x = np.asarray(inputs["x"], np.float32)
    edge_index = np.asarray(inputs["edge_index"])
    batch = np.asarray(inputs["batch"])
    Ws = [np.asarray(inputs[k], np.float32) for k in ("W_td1", "W_td2", "W_bu1", "W_bu2")]
    bs = [np.asarray(inputs[k], np.float32) for k in ("b_td1", "b_td2", "b_bu1", "b_bu2")]
    in_maps, meta = build_all_inputs(x, edge_index, batch, Ws, bs, cfg)
    nc = build_bass(meta)
    res = bass_utils.run_bass_kernel_spmd(
        nc, in_maps, core_ids=list(range(cfg["N_CORES"])), trace=trace)
    gpc = meta["part"]["gpc"]
    out = np.concatenate([res.results[c]["out"][:gpc] for c in range(cfg["N_CORES"])], axis=0)
    return out.astype(np.float32), res


def kernel(**inputs):
    out, _ = _run(inputs, FULL_CFG, trace=False)
    return out


# revision 20
# speedup vs baseline: 1.9605x; 1.9605x over previous
"""BiGCN (2-layer bidirectional GCN + global add pool) on 8 Trainium2 NeuronCores.

Strategy (hardcoded for the nn_BiGCN_graphcl problem shapes):
  - Nodes are sharded graph-aligned: core c owns graphs [128c, 128c+128) and
    their (contiguous, batch-sorted) node range, padded to a common NPC.
  - Per direction (td / bu), edges are assigned to the core owning their
    target node.  GCNConv is computed as
        out = dinv * (scatter_add(hn[src], dst) + hn) + b,   hn = dinv * (x @ W)
    so no per-edge scaling is needed on device.
  - The hn table ([8*NPC, 128] bf16) is AllGathered between layers; each core
    gathers rows for its edge shard with dma_gather (256B rows), builds a
    staircase one-hot with a DVE is_equal against an iota constant, and
    segment-sums on the TensorEngine into per-window (128-node) PSUM tiles.
  - The SPMD program is identical on all cores: all per-core variation lives
    in uploaded index/data tensors; run lengths are padded to the max across
    cores (pad slots gather row 0 of the block and carry dstloc=-1 so their
    one-hot column is zero).
  - Graph pooling is a second one-hot matmul into a [128 graphs, 128] PSUM
    tile; the host just concatenates the 8 per-core [128, 256] outputs.
"""

import math
import numpy as np
import ml_dtypes

BF16 = ml_dtypes.bfloat16

# ---------------------------------------------------------------- problem cfg
FULL_CFG = dict(
    N=100000, E=1600000, IN_FEATS=256, HIDDEN=128, OUT_FEATS=128,
    NUM_GRAPHS=1024, N_CORES=8, SW=12, NBLK=4,
)


def _round_up(x, m):
    return (x + m - 1) // m * m


# =====================================================================
# Host-side metadata construction
# =====================================================================

def build_partition(batch, cfg, deg_td=None, deg_bu=None):
    """Graph-aligned node partition. Returns dict with per-core node ranges.

    If degree arrays are given, each core's local node order is permuted so
    that per-window (128-node) degree sums cluster just under multiples of
    4*128 edges per (window, src-block) run, minimizing ceil-128 padding."""
    N, C, G = cfg["N"], cfg["N_CORES"], cfg["NUM_GRAPHS"]
    gpc = G // C  # graphs per core
    starts = np.searchsorted(batch, np.arange(0, G + 1, gpc))
    counts = np.diff(starts)
    NPC = max(128, _round_up(int(counts.max()), 128))
    W = NPC // 128
    node_core = np.searchsorted(starts[1:], np.arange(N), side="right")
    node_local = np.arange(N) - starts[node_core]

    if deg_td is not None:
        NBLK = cfg["NBLK"]
        MARGIN = 45 * NBLK  # leave room for cross-core/block-split variance
        for c in range(C):
            lo, hi = starts[c], starts[c + 1]
            cnt = hi - lo
            dt = deg_td[lo:hi].astype(np.int64)
            db = deg_bu[lo:hi].astype(np.int64)
            order = np.argsort(-(dt + db), kind="stable")
            tg_t = np.full(W, dt.sum() / W)
            tg_b = np.full(W, db.sum() / W)
            rem_t = tg_t.astype(np.float64).copy()
            rem_b = tg_b.astype(np.float64).copy()
            room = np.full(W, 128, np.int64)
            assign = np.empty(cnt, np.int64)
            for j in order:
                score = np.minimum(rem_t - dt[j], rem_b - db[j])
                score[room <= 0] = -np.inf
                w = int(np.argmax(score))
                assign[j] = w
                rem_t[w] -= dt[j]
                rem_b[w] -= db[j]
                room[w] -= 1
            # positions: window-major order
            slot_in_w = np.zeros(W, np.int64)
            newloc = np.empty(cnt, np.int64)
            for j in range(cnt):
                w = assign[j]
                newloc[j] = w * 128 + slot_in_w[w]
                slot_in_w[w] += 1
            node_local[lo:hi] = newloc

    table_row = node_core * NPC + node_local
    return dict(starts=starts, counts=counts, NPC=NPC, gpc=gpc,
                node_core=node_core.astype(np.int64),
                node_local=node_local.astype(np.int64),
                table_row=table_row.astype(np.int64))


def build_direction_meta(gather_nodes, target_nodes, part, cfg):
    """Build per-core gather index / dstloc arrays and the uniform group
    structure for one edge direction.

    gather_nodes[e]: node whose table row is gathered for edge e.
    target_nodes[e]: node receiving the contribution.
    """
    N, C = cfg["N"], cfg["N_CORES"]
    SW, NBLK = cfg["SW"], cfg["NBLK"]
    NPC = part["NPC"]
    W = NPC // 128
    NS = (W + SW - 1) // SW
    R = C * NPC

    deg = np.bincount(target_nodes, minlength=N).astype(np.float64) + 1.0

    # Unequal src blocks: size the first NBLK-1 blocks so the per-(window,
    # block) run count lands just under a multiple of 128 (minimizes ceil-128
    # padding); the remainder block is small.  All block sizes <= 32767
    # (int16 gather index range).
    mean_wsum = (deg.sum() - N) / (C * W)  # mean edges per 128-node window
    tgt_run = 128 * max(1, int(np.ceil((mean_wsum / NBLK + 45) / 128))) - 45
    share = min(tgt_run / max(mean_wsum, 1.0), 32767.0 / R)
    B = max(128, int(R * share))
    bounds = [min(i * B, R) for i in range(NBLK)] + [R]
    assert all(bounds[i + 1] - bounds[i] <= 32767 for i in range(NBLK))
    bounds_arr = np.array(bounds[1:-1])

    tr_g = part["table_row"][gather_nodes]
    t_core = part["node_core"][target_nodes]
    t_local = part["node_local"][target_nodes]
    lw = t_local // 128          # window
    dloc = t_local % 128         # position within window
    blk = np.searchsorted(bounds_arr, tr_g, side="right")
    idxv = tr_g - np.array(bounds[:-1])[blk]
    sup = lw // SW

    # per (core, s, b, w) counts -> uniform G
    keyW = (sup * NBLK + blk) * W + lw  # key within a core
    nkeys = NS * NBLK * W
    counts = np.zeros((C, nkeys), np.int64)
    for c in range(C):
        m = t_core == c
        counts[c] = np.bincount(keyW[m], minlength=nkeys)
    max_counts = counts.max(axis=0).reshape(NS, NBLK, W)

    G = np.ceil(max_counts / 128).astype(np.int64)  # groups per (s,b,w)
    # ensure every window has at least one group (psum must be written)
    for s in range(NS):
        w_lo, w_hi = s * SW, min((s + 1) * SW, W)
        for w in range(w_lo, w_hi):
            if G[s, :, w].sum() == 0:
                G[s, 0, w] = 1
        G[s, :, :w_lo] = 0
        G[s, :, w_hi:] = 0

    # structure: per (s,b): window col bases, totals
    struct = []
    for s in range(NS):
        w_lo, w_hi = s * SW, min((s + 1) * SW, W)
        for b in range(NBLK):
            g_list = G[s, b, w_lo:w_hi]
            base = np.concatenate([[0], np.cumsum(g_list)])
            struct.append(dict(s=s, b=b, w_lo=w_lo, w_hi=w_hi,
                               g_list=g_list, g_base=base,
                               G=int(g_list.sum())))
    # global column offsets
    offG = 0
    off16 = 0
    for sb in struct:
        sb["offG"] = offG
        sb["off16"] = off16
        offG += sb["G"]
        off16 += sb["G"] * 8  # 128 slots / 16
    CG = offG
    Gmax = max((sb["G"] for sb in struct), default=1)

    # per-edge slot assignment (per core)
    idx_all = np.zeros((C, 128, CG * 8), np.int16)
    dloc_all = np.full((C, 128, CG), -1.0, BF16)
    # precompute slot base for each (s,b,w): global slot start
    slot_base = np.zeros((NS, NBLK, W), np.int64)
    for sb in struct:
        s, b = sb["s"], sb["b"]
        for i, w in enumerate(range(sb["w_lo"], sb["w_hi"])):
            slot_base[s, b, w] = (sb["offG"] + sb["g_base"][i]) * 128

    for c in range(C):
        m = t_core == c
        k = keyW[m]
        order = np.argsort(k, kind="stable")
        ks = k[order]
        # rank within each run
        run_start = np.searchsorted(ks, np.arange(nkeys))
        rank = np.arange(len(ks)) - run_start[ks]
        sb_s = ks // (NBLK * W)
        sb_b = (ks // W) % NBLK
        sb_w = ks % W
        slot = slot_base[sb_s, sb_b, sb_w] + rank
        iv = idxv[m][order]
        dv = dloc[m][order]
        # idx wrapped layout: slot j -> (j%16, j//16), replicated x8
        prow = slot % 16
        pcol = slot // 16
        tmp = np.zeros((16, CG * 8), np.int16)
        tmp[prow, pcol] = iv.astype(np.int16)
        idx_all[c] = np.tile(tmp, (8, 1))
        dloc_all[c, slot % 128, slot // 128] = dv.astype(BF16)

    return dict(deg=deg, struct=struct, CG=CG, Gmax=Gmax, NS=NS, W=W,
                bounds=bounds, idx_all=idx_all, dloc_all=dloc_all)


def build_all_inputs(x, edge_index, batch, Ws, bs, cfg):
    """Produce per-core in_maps plus structural metadata."""
    C = cfg["N_CORES"]
    N = cfg["N"]
    src = np.asarray(edge_index[0])
    dst = np.asarray(edge_index[1])
    part = build_partition(batch, cfg,
                           deg_td=np.bincount(dst, minlength=N),
                           deg_bu=np.bincount(src, minlength=N))
    NPC = part["NPC"]
    W = NPC // 128

    td = build_direction_meta(src, dst, part, cfg)   # gather src row, scatter to dst
    bu = build_direction_meta(dst, src, part, cfg)   # reversed

    Gmax = max(td["Gmax"], bu["Gmax"])
    iota_rep = np.tile(np.arange(128, dtype=np.float32), Gmax)[None, :].repeat(128, 0).astype(BF16)

    # per-core tensors
    in_maps = []
    xT_full = np.ascontiguousarray(np.asarray(x).T)  # [IN, N]
    batch_np = np.asarray(batch)
    for c in range(C):
        lo, hi = part["starts"][c], part["starts"][c + 1]
        cnt = hi - lo
        li = part["node_local"][lo:hi]
        xT = np.zeros((cfg["IN_FEATS"], NPC), BF16)
        xT[:, li] = xT_full[:, lo:hi].astype(BF16)
        deg_t = np.ones((128, W), np.float32)
        deg_b = np.ones((128, W), np.float32)
        deg_t[li % 128, li // 128] = td["deg"][lo:hi].astype(np.float32)
        deg_b[li % 128, li // 128] = bu["deg"][lo:hi].astype(np.float32)
        bl = np.full((128, W), -1.0, BF16)
        bl[li % 128, li // 128] = (batch_np[lo:hi] - c * part["gpc"]).astype(BF16)
        im = dict(
            xT=xT, ident=np.eye(128, dtype=BF16),
            deg_td=deg_t, deg_bu=deg_b, batchloc=bl, iota_rep=iota_rep,
            idx_td=td["idx_all"][c], idx_bu=bu["idx_all"][c],
            dstloc_td=td["dloc_all"][c], dstloc_bu=bu["dloc_all"][c],
            W_td1=Ws[0].astype(BF16), W_bu1=Ws[2].astype(BF16),
            W_td2=Ws[1].astype(BF16), W_bu2=Ws[3].astype(BF16),
            b_td1=np.tile(bs[0][None, :], (128, 1)).astype(np.float32),
            b_td2=np.tile(bs[1][None, :], (128, 1)).astype(np.float32),
            b_bu1=np.tile(bs[2][None, :], (128, 1)).astype(np.float32),
            b_bu2=np.tile(bs[3][None, :], (128, 1)).astype(np.float32),
        )
        in_maps.append(im)
    meta = dict(part=part, td=td, bu=bu, Gmax=Gmax, NPC=NPC, W=W, cfg=cfg)
    return in_maps, meta


# =====================================================================
# Bass program
# =====================================================================

def build_bass(meta):
    import concourse.bacc as bacc
    import concourse.mybir as mybir
    import concourse.tile as tile

    cfg = meta["cfg"]
    C = cfg["N_CORES"]
    NPC, W, Gmax = meta["NPC"], meta["W"], meta["Gmax"]
    IN, HID = cfg["IN_FEATS"], cfg["HIDDEN"]
    NBLK = cfg["NBLK"]
    f32, bf16, i16 = mybir.dt.float32, mybir.dt.bfloat16, mybir.dt.int16

    nc = bacc.Bacc("TRN2", target_bir_lowering=False, debug=False, num_devices=C,
                   num_swdge_queues=4)

    # ---- I/O ----
    ten = {}
    def inp(name, shape, dt):
        ten[name] = nc.dram_tensor(name, shape, dt, kind="ExternalInput")
        return ten[name]

    inp("xT", [IN, NPC], bf16)
    inp("deg_td", [128, W], f32); inp("deg_bu", [128, W], f32)
    inp("batchloc", [128, W], bf16)
    inp("iota_rep", [128, Gmax * 128], bf16)
    inp("ident", [128, 128], bf16)
    for d in ("td", "bu"):
        m = meta[d]
        inp(f"idx_{d}", [128, m["CG"] * 8], i16)
        inp(f"dstloc_{d}", [128, m["CG"]], bf16)
        inp(f"W_{d}1", [IN, HID], bf16)
        inp(f"W_{d}2", [HID, HID], bf16)
        inp(f"b_{d}1", [128, HID], f32)
        inp(f"b_{d}2", [128, HID], f32)
    out_t = nc.dram_tensor("out", [128, 2 * HID], f32, kind="ExternalOutput")
    dbg = meta.get("dbg")
    if dbg:
        dbg_h1 = {d: nc.dram_tensor(f"dbg_h1_{d}", [NPC, HID], f32, kind="ExternalOutput")
                  for d in ("td", "bu")}
        dbg_m = {d: nc.dram_tensor(f"dbg_m_{d}", [NPC, HID], f32, kind="ExternalOutput")
                 for d in ("td", "bu")}

    # internal DRAM: AG inputs + tables
    ag_in, table = {}, {}
    for d in ("td", "bu"):
        for l in (1, 2):
            ag_in[d, l] = nc.dram_tensor(f"agin_{d}{l}", [NPC, HID], bf16, kind="Internal")
            table[d, l] = nc.dram_tensor(f"table_{d}{l}", [C * NPC, HID], bf16,
                                         kind="Internal", addr_space="Shared")

    rg = [list(range(C))]

    from contextlib import ExitStack
    with tile.TileContext(nc) as tc, ExitStack() as stack:
        def pool(name, bufs, space="SBUF"):
            return stack.enter_context(tc.tile_pool(name=name, bufs=bufs, space=space))

        const = pool("const", 1)
        xt_p = pool("xt", 6)
        hps_p = pool("hps", 2, "PSUM")      # table matmul psum
        hn_p = pool("hn", 4)                 # hn tiles to DRAM
        idx_p = pool("idx", 4)
        dl_p = pool("dl", 4)
        gat_p = pool("gat", 5)               # gathered edge tiles
        oh_p = pool("oh", 3)                 # one-hot tiles
        win_p = pool("win", 4, "PSUM")       # window psum, 4 windows/bank
        epi_p = pool("epi", 6)               # epilogue sbuf tiles
        h1_p = pool("h1", 4)
        t_p = pool("tt", 4)                  # transposes
        pool_ps = pool("plps", 1, "PSUM")    # pooling psum (held whole conv2)
        po_p = pool("po", 4)                 # pool one-hot
        outp = pool("outp", 1)

        # ---- constants in SBUF ----
        iota = const.tile([128, Gmax * 128], bf16, tag="iota")
        nc.sync.dma_start(iota[:], ten["iota_rep"][:])
        Wt = {}
        for d in ("td", "bu"):
            for l, k in ((1, IN), (2, HID)):
                chunks = []
                for kk in range(k // 128):
                    t = const.tile([128, HID], bf16, tag=f"W_{d}{l}_{kk}", name=f"W_{d}{l}_{kk}")
                    nc.sync.dma_start(t[:], ten[f"W_{d}{l}"][kk * 128:(kk + 1) * 128, :])
                    chunks.append(t)
                Wt[d, l] = chunks
        bt = {}
        for d in ("td", "bu"):
            for l in (1, 2):
                t = const.tile([128, HID], f32, tag=f"b_{d}{l}", name=f"bt_{d}{l}")
                nc.sync.dma_start(t[:], ten[f"b_{d}{l}"][:])
                bt[d, l] = t
        zrow = const.tile([1, 512], bf16, tag="zrow")
        nc.gpsimd.memset(zrow[:], 0.0)
        ident = const.tile([128, 128], bf16, tag="ident")
        nc.sync.dma_start(ident[:], ten["ident"][:])
        batchloc = const.tile([128, W], bf16, tag="batchloc")
        nc.sync.dma_start(batchloc[:], ten["batchloc"][:])

        dinv = {}
        for d in ("td", "bu"):
            degt = const.tile([128, W], f32, tag=f"deg_{d}", name=f"degt_{d}")
            nc.sync.dma_start(degt[:], ten[f"deg_{d}"][:])
            rec = const.tile([128, W], f32, tag=f"rec_{d}", name=f"rec_{d}")
            nc.vector.reciprocal(rec[:], degt[:])
            dv = const.tile([128, W], f32, tag=f"dinv_{d}", name=f"dinv_{d}")
            nc.scalar.activation(dv[:], rec[:], mybir.ActivationFunctionType.Sqrt)
            dinv[d] = dv

        # ---- phase A1: conv1 tables (both directions share xT loads) ----
        nK = IN // 128
        for w in range(W):
            xts = []
            for kk in range(nK):
                t = xt_p.tile([128, 128], bf16, tag="xt", name=f"xt_{w}_{kk}")
                nc.sync.dma_start(t[:], ten["xT"][kk * 128:(kk + 1) * 128,
                                                 w * 128:(w + 1) * 128])
                xts.append(t)
            for d in ("td", "bu"):
                hps = hps_p.tile([128, HID], f32, tag="hps")
                for kk in range(nK):
                    nc.tensor.matmul(hps[:], xts[kk][:], Wt[d, 1][kk][:],
                                     start=(kk == 0), stop=(kk == nK - 1))
                hn = hn_p.tile([128, HID], bf16, tag="hn")
                nc.vector.tensor_scalar_mul(hn[:], hps[:], dinv[d][:, w:w + 1])
                nc.sync.dma_start(ag_in[d, 1][w * 128:(w + 1) * 128, :], hn[:])

        for d in ("td", "bu"):
            nc.gpsimd.collective_compute(
                "AllGather", mybir.AluOpType.bypass, replica_groups=rg,
                ins=[ag_in[d, 1].ap()], outs=[table[d, 1].ap()])

        # ---- edge phase for one conv ----
        def edge_phase(d, l):
            m = meta[d]
            first_mm = {}
            last_mm = {}
            # find last (sb_idx, group) per window for stop flags
            for sbi, sb in enumerate(m["struct"]):
                for i, w in enumerate(range(sb["w_lo"], sb["w_hi"])):
                    if sb["g_list"][i] > 0:
                        last_mm[w] = (sbi, int(sb["g_base"][i]) + int(sb["g_list"][i]) - 1)
            quad_tiles = {}
            def win_ap(w):
                q = w // 4
                if q not in quad_tiles:
                    qt = win_p.tile([128, 512], f32, tag="win",
                                    name=f"win_{d}{l}_{q}")
                    nc.tensor.matmul(qt[:], zrow[0:1, 0:128], zrow[0:1, 0:512],
                                     start=True, stop=False, skip_group_check=True)
                    quad_tiles[q] = qt
                return quad_tiles[q][:, (w % 4) * 128:(w % 4 + 1) * 128]
            for sbi, sb in enumerate(m["struct"]):
                G = sb["G"]
                if G == 0:
                    continue
                it = idx_p.tile([128, G * 8], i16, tag="idx")
                nc.sync.dma_start(it[:], ten[f"idx_{d}"][:, sb["off16"]:sb["off16"] + G * 8])
                dlt = dl_p.tile([128, G], bf16, tag="dl")
                nc.sync.dma_start(dlt[:], ten[f"dstloc_{d}"][:, sb["offG"]:sb["offG"] + G])
                gt = gat_p.tile([128, G, 128], bf16, tag="gat")
                blk = table[d, l][m["bounds"][sb["b"]]:m["bounds"][sb["b"] + 1], :]
                nc.gpsimd.dma_gather(gt[:], blk, it[:], num_idxs=G * 128,
                                     num_idxs_reg=G * 128, elem_size=HID,
                                     single_packet=False, queue_num=sbi % 4)
                oh = oh_p.tile([128, G * 128], bf16, tag="oh")
                nc.vector.tensor_tensor(
                    out=oh[:],
                    in0=dlt[:].rearrange("p (g o) -> p g o", o=1).to_broadcast([128, G, 128]),
                    in1=iota[:, :G * 128].rearrange("p (g f) -> p g f", f=128),
                    op=mybir.AluOpType.is_equal)
                for i, w in enumerate(range(sb["w_lo"], sb["w_hi"])):
                    gl = int(sb["g_list"][i])
                    if gl == 0:
                        continue
                    pt = win_ap(w)
                    gb = int(sb["g_base"][i])
                    for g in range(gb, gb + gl):
                        nc.tensor.matmul(
                            pt[:], oh[:, g * 128:(g + 1) * 128], gt[:, g, :],
                            start=False, stop=(last_mm[w] == (sbi, g)),
                            skip_group_check=True)
                # epilogues for completed supers: after last block of super
                if sb["b"] == NBLK - 1:
                    for w in range(sb["w_lo"], sb["w_hi"]):
                        epilogue(d, l, w, win_ap(w))
                    quad_tiles.clear()

        def epilogue(d, l, w, pt):
            hn = hn_p.tile([128, HID], bf16, tag="hn_ep")
            nc.sync.dma_start(hn[:], ag_in[d, l][w * 128:(w + 1) * 128, :])
            o1 = epi_p.tile([128, HID], f32, tag="o1")
            nc.vector.scalar_tensor_tensor(
                out=o1[:], in0=pt[:], scalar=dinv[d][:, w:w + 1], in1=bt[d, l][:],
                op0=mybir.AluOpType.mult, op1=mybir.AluOpType.add)
            o2 = epi_p.tile([128, HID], bf16, tag="o2")
            nc.vector.scalar_tensor_tensor(
                out=o2[:], in0=hn[:], scalar=dinv[d][:, w:w + 1], in1=o1[:],
                op0=mybir.AluOpType.mult, op1=mybir.AluOpType.add)
            if dbg and l == 1:
                mf = epi_p.tile([128, HID], f32, tag="mf")
                nc.vector.tensor_copy(mf[:], pt[:])
                nc.sync.dma_start(dbg_m[d][w * 128:(w + 1) * 128, :], mf[:])
            if l == 1:
                h1 = h1_p.tile([128, HID], bf16, tag="h1")
                nc.scalar.activation(h1[:], o2[:], mybir.ActivationFunctionType.Relu)
                if dbg:
                    h1f = epi_p.tile([128, HID], f32, tag="h1f")
                    nc.vector.tensor_copy(h1f[:], h1[:])
                    nc.sync.dma_start(dbg_h1[d][w * 128:(w + 1) * 128, :], h1f[:])
                tps = hps_p.tile([128, HID], bf16, tag="hps", name=f"tps_{d}_{w}")
                nc.tensor.transpose(tps[:], h1[:], ident[:])
                h1T = t_p.tile([128, HID], bf16, tag="h1T")
                nc.vector.tensor_copy(h1T[:], tps[:])
                h2 = hps_p.tile([128, HID], f32, tag="hps")
                nc.tensor.matmul(h2[:], h1T[:], Wt[d, 2][0][:], start=True, stop=True)
                hn2 = hn_p.tile([128, HID], bf16, tag="hn2")
                nc.vector.tensor_scalar_mul(hn2[:], h2[:], dinv[d][:, w:w + 1])
                nc.sync.dma_start(ag_in[d, 2][w * 128:(w + 1) * 128, :], hn2[:])
            else:
                po = po_p.tile([128, 128], bf16, tag="po")
                nc.vector.tensor_tensor(
                    out=po[:],
                    in0=batchloc[:, w:w + 1].to_broadcast([128, 128]),
                    in1=iota[:, :128],
                    op=mybir.AluOpType.is_equal)
                off = 0 if d == "td" else HID
                nc.tensor.matmul(pool_psum_t[:, off:off + HID], po[:], o2[:],
                                 start=False, stop=(w == W - 1),
                                 skip_group_check=True)

        # conv1 td -> AG2 td; conv1 bu -> AG2 bu; conv2 td; conv2 bu
        edge_phase("td", 1)
        nc.gpsimd.collective_compute(
            "AllGather", mybir.AluOpType.bypass, replica_groups=rg,
            ins=[ag_in["td", 2].ap()], outs=[table["td", 2].ap()])
        edge_phase("bu", 1)
        nc.gpsimd.collective_compute(
            "AllGather", mybir.AluOpType.bypass, replica_groups=rg,
            ins=[ag_in["bu", 2].ap()], outs=[table["bu", 2].ap()])
        pool_psum_t = pool_ps.tile([128, 2 * HID], f32, tag="pool", name="pool_psum_t")
        nc.tensor.matmul(pool_psum_t[:], zrow[0:1, 0:128], zrow[0:1, 0:2 * HID],
                         start=True, stop=False, skip_group_check=True)
        edge_phase("td", 2)
        edge_phase("bu", 2)

        outsb = outp.tile([128, 2 * HID], f32, tag="out")
        nc.vector.tensor_copy(outsb[:], pool_psum_t[:])
        nc.sync.dma_start(out_t[:], outsb[:])

    nc.compile()
    return nc


# =====================================================================
# Entry point
# =====================================================================

def _run(inputs, cfg, trace=False):
    from concourse import bass_utils
    x = np.asarray(inputs["x"], np.float32)
    edge_index = np.asarray(inputs["edge_index"])
    batch = np.asarray(inputs["batch"])
    Ws = [np.asarray(inputs[k], np.float32) for k in ("W_td1", "W_td2", "W_bu1", "W_bu2")]
    bs = [np.asarray(inputs[k], np.float32) for k in ("b_td1", "b_td2", "b_bu1", "b_bu2")]
    in_maps, meta = build_all_inputs(x, edge_index, batch, Ws, bs, cfg)
    nc = build_bass(meta)
    res = bass_utils.run_bass_kernel_spmd(
        nc, in_maps, core_ids=list(range(cfg["N_CORES"])), trace=trace)
    gpc = meta["part"]["gpc"]
    out = np.concatenate([res.results[c]["out"][:gpc] for c in range(cfg["N_CORES"])], axis=0)
    return out.astype(np.float32), res


def kernel(**inputs):
    out, _ = _run(inputs, FULL_CFG, trace=False)
    return out


# revision 21
# speedup vs baseline: 1.9621x; 1.0008x over previous
"""BiGCN (2-layer bidirectional GCN + global add pool) on 8 Trainium2 NeuronCores.

Strategy (hardcoded for the nn_BiGCN_graphcl problem shapes):
  - Nodes are sharded graph-aligned: core c owns graphs [128c, 128c+128) and
    their (contiguous, batch-sorted) node range, padded to a common NPC.
  - Per direction (td / bu), edges are assigned to the core owning their
    target node.  GCNConv is computed as
        out = dinv * (scatter_add(hn[src], dst) + hn) + b,   hn = dinv * (x @ W)
    so no per-edge scaling is needed on device.
  - The hn table ([8*NPC, 128] bf16) is AllGathered between layers; each core
    gathers rows for its edge shard with dma_gather (256B rows), builds a
    staircase one-hot with a DVE is_equal against an iota constant, and
    segment-sums on the TensorEngine into per-window (128-node) PSUM tiles.
  - The SPMD program is identical on all cores: all per-core variation lives
    in uploaded index/data tensors; run lengths are padded to the max across
    cores (pad slots gather row 0 of the block and carry dstloc=-1 so their
    one-hot column is zero).
  - Graph pooling is a second one-hot matmul into a [128 graphs, 128] PSUM
    tile; the host just concatenates the 8 per-core [128, 256] outputs.
"""

import math
import numpy as np
import ml_dtypes

BF16 = ml_dtypes.bfloat16

# ---------------------------------------------------------------- problem cfg
FULL_CFG = dict(
    N=100000, E=1600000, IN_FEATS=256, HIDDEN=128, OUT_FEATS=128,
    NUM_GRAPHS=1024, N_CORES=8, SW=8, NBLK=4,
)


def _round_up(x, m):
    return (x + m - 1) // m * m


# =====================================================================
# Host-side metadata construction
# =====================================================================

def build_partition(batch, cfg, deg_td=None, deg_bu=None):
    """Graph-aligned node partition. Returns dict with per-core node ranges.

    If degree arrays are given, each core's local node order is permuted so
    that per-window (128-node) degree sums cluster just under multiples of
    4*128 edges per (window, src-block) run, minimizing ceil-128 padding."""
    N, C, G = cfg["N"], cfg["N_CORES"], cfg["NUM_GRAPHS"]
    gpc = G // C  # graphs per core
    starts = np.searchsorted(batch, np.arange(0, G + 1, gpc))
    counts = np.diff(starts)
    NPC = max(128, _round_up(int(counts.max()), 128))
    W = NPC // 128
    node_core = np.searchsorted(starts[1:], np.arange(N), side="right")
    node_local = np.arange(N) - starts[node_core]

    if deg_td is not None:
        NBLK = cfg["NBLK"]
        MARGIN = 45 * NBLK  # leave room for cross-core/block-split variance
        for c in range(C):
            lo, hi = starts[c], starts[c + 1]
            cnt = hi - lo
            dt = deg_td[lo:hi].astype(np.int64)
            db = deg_bu[lo:hi].astype(np.int64)
            order = np.argsort(-(dt + db), kind="stable")
            tg_t = np.full(W, dt.sum() / W)
            tg_b = np.full(W, db.sum() / W)
            rem_t = tg_t.astype(np.float64).copy()
            rem_b = tg_b.astype(np.float64).copy()
            room = np.full(W, 128, np.int64)
            assign = np.empty(cnt, np.int64)
            for j in order:
                score = np.minimum(rem_t - dt[j], rem_b - db[j])
                score[room <= 0] = -np.inf
                w = int(np.argmax(score))
                assign[j] = w
                rem_t[w] -= dt[j]
                rem_b[w] -= db[j]
                room[w] -= 1
            # positions: window-major order
            slot_in_w = np.zeros(W, np.int64)
            newloc = np.empty(cnt, np.int64)
            for j in range(cnt):
                w = assign[j]
                newloc[j] = w * 128 + slot_in_w[w]
                slot_in_w[w] += 1
            node_local[lo:hi] = newloc

    table_row = node_core * NPC + node_local
    return dict(starts=starts, counts=counts, NPC=NPC, gpc=gpc,
                node_core=node_core.astype(np.int64),
                node_local=node_local.astype(np.int64),
                table_row=table_row.astype(np.int64))


def build_direction_meta(gather_nodes, target_nodes, part, cfg):
    """Build per-core gather index / dstloc arrays and the uniform group
    structure for one edge direction.

    gather_nodes[e]: node whose table row is gathered for edge e.
    target_nodes[e]: node receiving the contribution.
    """
    N, C = cfg["N"], cfg["N_CORES"]
    SW, NBLK = cfg["SW"], cfg["NBLK"]
    NPC = part["NPC"]
    W = NPC // 128
    NS = (W + SW - 1) // SW
    R = C * NPC

    deg = np.bincount(target_nodes, minlength=N).astype(np.float64) + 1.0

    # Unequal src blocks: size the first NBLK-1 blocks so the per-(window,
    # block) run count lands just under a multiple of 128 (minimizes ceil-128
    # padding); the remainder block is small.  All block sizes <= 32767
    # (int16 gather index range).
    mean_wsum = (deg.sum() - N) / (C * W)  # mean edges per 128-node window
    tgt_run = 128 * max(1, int(np.ceil((mean_wsum / NBLK + 45) / 128))) - 45
    share = min(tgt_run / max(mean_wsum, 1.0), 32767.0 / R)
    B = max(128, int(R * share))
    bounds = [min(i * B, R) for i in range(NBLK)] + [R]
    assert all(bounds[i + 1] - bounds[i] <= 32767 for i in range(NBLK))
    bounds_arr = np.array(bounds[1:-1])

    tr_g = part["table_row"][gather_nodes]
    t_core = part["node_core"][target_nodes]
    t_local = part["node_local"][target_nodes]
    lw = t_local // 128          # window
    dloc = t_local % 128         # position within window
    blk = np.searchsorted(bounds_arr, tr_g, side="right")
    idxv = tr_g - np.array(bounds[:-1])[blk]
    sup = lw // SW

    # per (core, s, b, w) counts -> uniform G
    keyW = (sup * NBLK + blk) * W + lw  # key within a core
    nkeys = NS * NBLK * W
    counts = np.zeros((C, nkeys), np.int64)
    for c in range(C):
        m = t_core == c
        counts[c] = np.bincount(keyW[m], minlength=nkeys)
    max_counts = counts.max(axis=0).reshape(NS, NBLK, W)

    G = np.ceil(max_counts / 128).astype(np.int64)  # groups per (s,b,w)
    # ensure every window has at least one group (psum must be written)
    for s in range(NS):
        w_lo, w_hi = s * SW, min((s + 1) * SW, W)
        for w in range(w_lo, w_hi):
            if G[s, :, w].sum() == 0:
                G[s, 0, w] = 1
        G[s, :, :w_lo] = 0
        G[s, :, w_hi:] = 0

    # structure: per (s,b): window col bases, totals
    struct = []
    for s in range(NS):
        w_lo, w_hi = s * SW, min((s + 1) * SW, W)
        for b in range(NBLK):
            g_list = G[s, b, w_lo:w_hi]
            base = np.concatenate([[0], np.cumsum(g_list)])
            struct.append(dict(s=s, b=b, w_lo=w_lo, w_hi=w_hi,
                               g_list=g_list, g_base=base,
                               G=int(g_list.sum())))
    # global column offsets
    offG = 0
    off16 = 0
    for sb in struct:
        sb["offG"] = offG
        sb["off16"] = off16
        offG += sb["G"]
        off16 += sb["G"] * 8  # 128 slots / 16
    CG = offG
    Gmax = max((sb["G"] for sb in struct), default=1)

    # per-edge slot assignment (per core)
    idx_all = np.zeros((C, 128, CG * 8), np.int16)
    dloc_all = np.full((C, 128, CG), -1.0, BF16)
    # precompute slot base for each (s,b,w): global slot start
    slot_base = np.zeros((NS, NBLK, W), np.int64)
    for sb in struct:
        s, b = sb["s"], sb["b"]
        for i, w in enumerate(range(sb["w_lo"], sb["w_hi"])):
            slot_base[s, b, w] = (sb["offG"] + sb["g_base"][i]) * 128

    for c in range(C):
        m = t_core == c
        k = keyW[m]
        order = np.argsort(k, kind="stable")
        ks = k[order]
        # rank within each run
        run_start = np.searchsorted(ks, np.arange(nkeys))
        rank = np.arange(len(ks)) - run_start[ks]
        sb_s = ks // (NBLK * W)
        sb_b = (ks // W) % NBLK
        sb_w = ks % W
        slot = slot_base[sb_s, sb_b, sb_w] + rank
        iv = idxv[m][order]
        dv = dloc[m][order]
        # idx wrapped layout: slot j -> (j%16, j//16), replicated x8
        prow = slot % 16
        pcol = slot // 16
        tmp = np.zeros((16, CG * 8), np.int16)
        tmp[prow, pcol] = iv.astype(np.int16)
        idx_all[c] = np.tile(tmp, (8, 1))
        dloc_all[c, slot % 128, slot // 128] = dv.astype(BF16)

    return dict(deg=deg, struct=struct, CG=CG, Gmax=Gmax, NS=NS, W=W,
                bounds=bounds, idx_all=idx_all, dloc_all=dloc_all)


def build_all_inputs(x, edge_index, batch, Ws, bs, cfg):
    """Produce per-core in_maps plus structural metadata."""
    C = cfg["N_CORES"]
    N = cfg["N"]
    src = np.asarray(edge_index[0])
    dst = np.asarray(edge_index[1])
    part = build_partition(batch, cfg,
                           deg_td=np.bincount(dst, minlength=N),
                           deg_bu=np.bincount(src, minlength=N))
    NPC = part["NPC"]
    W = NPC // 128

    td = build_direction_meta(src, dst, part, cfg)   # gather src row, scatter to dst
    bu = build_direction_meta(dst, src, part, cfg)   # reversed

    Gmax = max(td["Gmax"], bu["Gmax"])
    iota_rep = np.tile(np.arange(128, dtype=np.float32), Gmax)[None, :].repeat(128, 0).astype(BF16)

    # per-core tensors
    in_maps = []
    xT_full = np.ascontiguousarray(np.asarray(x).T)  # [IN, N]
    batch_np = np.asarray(batch)
    for c in range(C):
        lo, hi = part["starts"][c], part["starts"][c + 1]
        cnt = hi - lo
        li = part["node_local"][lo:hi]
        xT = np.zeros((cfg["IN_FEATS"], NPC), BF16)
        xT[:, li] = xT_full[:, lo:hi].astype(BF16)
        deg_t = np.ones((128, W), np.float32)
        deg_b = np.ones((128, W), np.float32)
        deg_t[li % 128, li // 128] = td["deg"][lo:hi].astype(np.float32)
        deg_b[li % 128, li // 128] = bu["deg"][lo:hi].astype(np.float32)
        bl = np.full((128, W), -1.0, BF16)
        bl[li % 128, li // 128] = (batch_np[lo:hi] - c * part["gpc"]).astype(BF16)
        im = dict(
            xT=xT, ident=np.eye(128, dtype=BF16),
            deg_td=deg_t, deg_bu=deg_b, batchloc=bl, iota_rep=iota_rep,
            idx_td=td["idx_all"][c], idx_bu=bu["idx_all"][c],
            dstloc_td=td["dloc_all"][c], dstloc_bu=bu["dloc_all"][c],
            W_td1=Ws[0].astype(BF16), W_bu1=Ws[2].astype(BF16),
            W_td2=Ws[1].astype(BF16), W_bu2=Ws[3].astype(BF16),
            b_td1=np.tile(bs[0][None, :], (128, 1)).astype(np.float32),
            b_td2=np.tile(bs[1][None, :], (128, 1)).astype(np.float32),
            b_bu1=np.tile(bs[2][None, :], (128, 1)).astype(np.float32),
            b_bu2=np.tile(bs[3][None, :], (128, 1)).astype(np.float32),
        )
        in_maps.append(im)
    meta = dict(part=part, td=td, bu=bu, Gmax=Gmax, NPC=NPC, W=W, cfg=cfg)
    return in_maps, meta


# =====================================================================
# Bass program
# =====================================================================

def build_bass(meta):
    import concourse.bacc as bacc
    import concourse.mybir as mybir
    import concourse.tile as tile

    cfg = meta["cfg"]
    C = cfg["N_CORES"]
    NPC, W, Gmax = meta["NPC"], meta["W"], meta["Gmax"]
    IN, HID = cfg["IN_FEATS"], cfg["HIDDEN"]
    NBLK = cfg["NBLK"]
    f32, bf16, i16 = mybir.dt.float32, mybir.dt.bfloat16, mybir.dt.int16

    nc = bacc.Bacc("TRN2", target_bir_lowering=False, debug=False, num_devices=C,
                   num_swdge_queues=4)

    # ---- I/O ----
    ten = {}
    def inp(name, shape, dt):
        ten[name] = nc.dram_tensor(name, shape, dt, kind="ExternalInput")
        return ten[name]

    inp("xT", [IN, NPC], bf16)
    inp("deg_td", [128, W], f32); inp("deg_bu", [128, W], f32)
    inp("batchloc", [128, W], bf16)
    inp("iota_rep", [128, Gmax * 128], bf16)
    inp("ident", [128, 128], bf16)
    for d in ("td", "bu"):
        m = meta[d]
        inp(f"idx_{d}", [128, m["CG"] * 8], i16)
        inp(f"dstloc_{d}", [128, m["CG"]], bf16)
        inp(f"W_{d}1", [IN, HID], bf16)
        inp(f"W_{d}2", [HID, HID], bf16)
        inp(f"b_{d}1", [128, HID], f32)
        inp(f"b_{d}2", [128, HID], f32)
    out_t = nc.dram_tensor("out", [128, 2 * HID], f32, kind="ExternalOutput")
    dbg = meta.get("dbg")
    if dbg:
        dbg_h1 = {d: nc.dram_tensor(f"dbg_h1_{d}", [NPC, HID], f32, kind="ExternalOutput")
                  for d in ("td", "bu")}
        dbg_m = {d: nc.dram_tensor(f"dbg_m_{d}", [NPC, HID], f32, kind="ExternalOutput")
                 for d in ("td", "bu")}

    # internal DRAM: AG inputs + tables
    ag_in, table = {}, {}
    for d in ("td", "bu"):
        for l in (1, 2):
            ag_in[d, l] = nc.dram_tensor(f"agin_{d}{l}", [NPC, HID], bf16, kind="Internal")
            table[d, l] = nc.dram_tensor(f"table_{d}{l}", [C * NPC, HID], bf16,
                                         kind="Internal", addr_space="Shared")

    rg = [list(range(C))]

    from contextlib import ExitStack
    with tile.TileContext(nc) as tc, ExitStack() as stack:
        def pool(name, bufs, space="SBUF"):
            return stack.enter_context(tc.tile_pool(name=name, bufs=bufs, space=space))

        const = pool("const", 1)
        xt_p = pool("xt", 6)
        hn_p = pool("hn", 4)                 # hn tiles to DRAM
        idx_p = pool("idx", 4)
        dl_p = pool("dl", 4)
        gat_p = pool("gat", 5)               # gathered edge tiles
        oh_p = pool("oh", 3)                 # one-hot tiles
        win_p = pool("win", 6, "PSUM")       # window psum, 4 windows/bank
        epi_p = pool("epi", 6)               # epilogue sbuf tiles
        h1_p = pool("h1", 4)
        t_p = pool("tt", 4)                  # transposes
        po_p = pool("po", 4)                 # pool one-hot
        outp = pool("outp", 1)
        hps_cm = tc.tile_pool(name="hps", bufs=2, space="PSUM")
        hps_p = hps_cm.__enter__()

        # ---- constants in SBUF ----
        iota = const.tile([128, Gmax * 128], bf16, tag="iota")
        nc.sync.dma_start(iota[:], ten["iota_rep"][:])
        Wt = {}
        for d in ("td", "bu"):
            for l, k in ((1, IN), (2, HID)):
                chunks = []
                for kk in range(k // 128):
                    t = const.tile([128, HID], bf16, tag=f"W_{d}{l}_{kk}", name=f"W_{d}{l}_{kk}")
                    nc.sync.dma_start(t[:], ten[f"W_{d}{l}"][kk * 128:(kk + 1) * 128, :])
                    chunks.append(t)
                Wt[d, l] = chunks
        bt = {}
        for d in ("td", "bu"):
            for l in (1, 2):
                t = const.tile([128, HID], f32, tag=f"b_{d}{l}", name=f"bt_{d}{l}")
                nc.sync.dma_start(t[:], ten[f"b_{d}{l}"][:])
                bt[d, l] = t
        zrow = const.tile([1, 512], bf16, tag="zrow")
        nc.gpsimd.memset(zrow[:], 0.0)
        ident = const.tile([128, 128], bf16, tag="ident")
        nc.sync.dma_start(ident[:], ten["ident"][:])
        batchloc = const.tile([128, W], bf16, tag="batchloc")
        nc.sync.dma_start(batchloc[:], ten["batchloc"][:])

        dinv = {}
        for d in ("td", "bu"):
            degt = const.tile([128, W], f32, tag=f"deg_{d}", name=f"degt_{d}")
            nc.sync.dma_start(degt[:], ten[f"deg_{d}"][:])
            rec = const.tile([128, W], f32, tag=f"rec_{d}", name=f"rec_{d}")
            nc.vector.reciprocal(rec[:], degt[:])
            dv = const.tile([128, W], f32, tag=f"dinv_{d}", name=f"dinv_{d}")
            nc.scalar.activation(dv[:], rec[:], mybir.ActivationFunctionType.Sqrt)
            dinv[d] = dv

        # ---- phase A1: conv1 tables (both directions share xT loads) ----
        nK = IN // 128
        for w in range(W):
            xts = []
            for kk in range(nK):
                t = xt_p.tile([128, 128], bf16, tag="xt", name=f"xt_{w}_{kk}")
                nc.sync.dma_start(t[:], ten["xT"][kk * 128:(kk + 1) * 128,
                                                 w * 128:(w + 1) * 128])
                xts.append(t)
            for d in ("td", "bu"):
                hps = hps_p.tile([128, HID], f32, tag="hps")
                for kk in range(nK):
                    nc.tensor.matmul(hps[:], xts[kk][:], Wt[d, 1][kk][:],
                                     start=(kk == 0), stop=(kk == nK - 1))
                hn = hn_p.tile([128, HID], bf16, tag="hn")
                nc.vector.tensor_scalar_mul(hn[:], hps[:], dinv[d][:, w:w + 1])
                nc.sync.dma_start(ag_in[d, 1][w * 128:(w + 1) * 128, :], hn[:])

        for d in ("td", "bu"):
            nc.gpsimd.collective_compute(
                "AllGather", mybir.AluOpType.bypass, replica_groups=rg,
                ins=[ag_in[d, 1].ap()], outs=[table[d, 1].ap()])

        # ---- edge phase for one conv ----
        def edge_phase(d, l):
            m = meta[d]
            first_mm = {}
            last_mm = {}
            # find last (sb_idx, group) per window for stop flags
            for sbi, sb in enumerate(m["struct"]):
                for i, w in enumerate(range(sb["w_lo"], sb["w_hi"])):
                    if sb["g_list"][i] > 0:
                        last_mm[w] = (sbi, int(sb["g_base"][i]) + int(sb["g_list"][i]) - 1)
            quad_tiles = {}
            def win_ap(w):
                q = w // 4
                if q not in quad_tiles:
                    qt = win_p.tile([128, 512], f32, tag="win",
                                    name=f"win_{d}{l}_{q}")
                    nc.tensor.matmul(qt[:], zrow[0:1, 0:128], zrow[0:1, 0:512],
                                     start=True, stop=False, skip_group_check=True)
                    quad_tiles[q] = qt
                return quad_tiles[q][:, (w % 4) * 128:(w % 4 + 1) * 128]
            for sbi, sb in enumerate(m["struct"]):
                G = sb["G"]
                if G == 0:
                    continue
                it = idx_p.tile([128, G * 8], i16, tag="idx")
                nc.sync.dma_start(it[:], ten[f"idx_{d}"][:, sb["off16"]:sb["off16"] + G * 8])
                dlt = dl_p.tile([128, G], bf16, tag="dl")
                nc.sync.dma_start(dlt[:], ten[f"dstloc_{d}"][:, sb["offG"]:sb["offG"] + G])
                gt = gat_p.tile([128, G, 128], bf16, tag="gat")
                blk = table[d, l][m["bounds"][sb["b"]]:m["bounds"][sb["b"] + 1], :]
                qn[0] += 1
                nc.gpsimd.dma_gather(gt[:], blk, it[:], num_idxs=G * 128,
                                     num_idxs_reg=G * 128, elem_size=HID,
                                     single_packet=False, queue_num=qn[0] % 4)
                oh = oh_p.tile([128, G * 128], bf16, tag="oh")
                nc.vector.tensor_tensor(
                    out=oh[:],
                    in0=dlt[:].rearrange("p (g o) -> p g o", o=1).to_broadcast([128, G, 128]),
                    in1=iota[:, :G * 128].rearrange("p (g f) -> p g f", f=128),
                    op=mybir.AluOpType.is_equal)
                for i, w in enumerate(range(sb["w_lo"], sb["w_hi"])):
                    gl = int(sb["g_list"][i])
                    if gl == 0:
                        continue
                    pt = win_ap(w)
                    gb = int(sb["g_base"][i])
                    for g in range(gb, gb + gl):
                        nc.tensor.matmul(
                            pt[:], oh[:, g * 128:(g + 1) * 128], gt[:, g, :],
                            start=False, stop=(last_mm[w] == (sbi, g)),
                            skip_group_check=True)
                # epilogues for completed supers: after last block of super
                if sb["b"] == NBLK - 1:
                    for w in range(sb["w_lo"], sb["w_hi"]):
                        epilogue(d, l, w, win_ap(w))
                    quad_tiles.clear()
                yield

        def epilogue(d, l, w, pt):
            hn = hn_p.tile([128, HID], bf16, tag="hn_ep")
            nc.sync.dma_start(hn[:], ag_in[d, l][w * 128:(w + 1) * 128, :])
            o1 = epi_p.tile([128, HID], f32, tag="o1")
            nc.vector.scalar_tensor_tensor(
                out=o1[:], in0=pt[:], scalar=dinv[d][:, w:w + 1], in1=bt[d, l][:],
                op0=mybir.AluOpType.mult, op1=mybir.AluOpType.add)
            o2 = epi_p.tile([128, HID], bf16, tag="o2")
            nc.vector.scalar_tensor_tensor(
                out=o2[:], in0=hn[:], scalar=dinv[d][:, w:w + 1], in1=o1[:],
                op0=mybir.AluOpType.mult, op1=mybir.AluOpType.add)
            if dbg and l == 1:
                mf = epi_p.tile([128, HID], f32, tag="mf")
                nc.vector.tensor_copy(mf[:], pt[:])
                nc.sync.dma_start(dbg_m[d][w * 128:(w + 1) * 128, :], mf[:])
            if l == 1:
                h1 = h1_p.tile([128, HID], bf16, tag="h1")
                nc.scalar.activation(h1[:], o2[:], mybir.ActivationFunctionType.Relu)
                if dbg:
                    h1f = epi_p.tile([128, HID], f32, tag="h1f")
                    nc.vector.tensor_copy(h1f[:], h1[:])
                    nc.sync.dma_start(dbg_h1[d][w * 128:(w + 1) * 128, :], h1f[:])
                tps = hps_p.tile([128, HID], bf16, tag="hps", name=f"tps_{d}_{w}")
                nc.tensor.transpose(tps[:], h1[:], ident[:])
                h1T = t_p.tile([128, HID], bf16, tag="h1T")
                nc.vector.tensor_copy(h1T[:], tps[:])
                h2 = hps_p.tile([128, HID], f32, tag="hps")
                nc.tensor.matmul(h2[:], h1T[:], Wt[d, 2][0][:], start=True, stop=True)
                hn2 = hn_p.tile([128, HID], bf16, tag="hn2")
                nc.vector.tensor_scalar_mul(hn2[:], h2[:], dinv[d][:, w:w + 1])
                nc.sync.dma_start(ag_in[d, 2][w * 128:(w + 1) * 128, :], hn2[:])
            else:
                po = po_p.tile([128, 128], bf16, tag="po")
                nc.vector.tensor_tensor(
                    out=po[:],
                    in0=batchloc[:, w:w + 1].to_broadcast([128, 128]),
                    in1=iota[:, :128],
                    op=mybir.AluOpType.is_equal)
                off = 0 if d == "td" else HID
                nc.tensor.matmul(pool_psum_t[:, off:off + HID], po[:], o2[:],
                                 start=False, stop=(w == W - 1),
                                 skip_group_check=True)

        qn = [0]

        def run_layer(l):
            gens = {"td": edge_phase("td", l), "bu": edge_phase("bu", l)}
            done = {"td": False, "bu": False}
            while not all(done.values()):
                for d in ("td", "bu"):
                    if done[d]:
                        continue
                    try:
                        next(gens[d])
                    except StopIteration:
                        done[d] = True
                        if l == 1:
                            nc.gpsimd.collective_compute(
                                "AllGather", mybir.AluOpType.bypass,
                                replica_groups=rg,
                                ins=[ag_in[d, 2].ap()], outs=[table[d, 2].ap()])

        run_layer(1)
        hps_cm.__exit__(None, None, None)
        pool_ps = stack.enter_context(tc.tile_pool(name="plps", bufs=1, space="PSUM"))
        pool_psum_t = pool_ps.tile([128, 2 * HID], f32, tag="pool", name="pool_psum_t")
        nc.tensor.matmul(pool_psum_t[:], zrow[0:1, 0:128], zrow[0:1, 0:2 * HID],
                         start=True, stop=False, skip_group_check=True)
        run_layer(2)

        outsb = outp.tile([128, 2 * HID], f32, tag="out")
        nc.vector.tensor_copy(outsb[:], pool_psum_t[:])
        nc.sync.dma_start(out_t[:], outsb[:])

    nc.compile()
    return nc


# =====================================================================
# Entry point
# =====================================================================

def _run(inputs, cfg, trace=False):
    from concourse import bass_utils
    x = np.asarray(inputs["x"], np.float32)
    edge_index = np.asarray(inputs["edge_index"])
    batch = np.asarray(inputs["batch"])
    Ws = [np.asarray(inputs[k], np.float32) for k in ("W_td1", "W_td2", "W_bu1", "W_bu2")]
    bs = [np.asarray(inputs[k], np.float32) for k in ("b_td1", "b_td2", "b_bu1", "b_bu2")]
    in_maps, meta = build_all_inputs(x, edge_index, batch, Ws, bs, cfg)
    nc = build_bass(meta)
    res = bass_utils.run_bass_kernel_spmd(
        nc, in_maps, core_ids=list(range(cfg["N_CORES"])), trace=trace)
    gpc = meta["part"]["gpc"]
    out = np.concatenate([res.results[c]["out"][:gpc] for c in range(cfg["N_CORES"])], axis=0)
    return out.astype(np.float32), res


def kernel(**inputs):
    out, _ = _run(inputs, FULL_CFG, trace=False)
    return out
